# revision 33
# baseline (speedup 1.0000x reference)
"""Distributed multi-head attention for 8 trn2 NeuronCores.

Strategy (Ulysses-style head-sharding):
  - Every core receives the full activations pre-transposed/cast host-side:
    xT [C, B*N] bf16. Heads are sharded 2-per-core for QKV + attention
    (Megatron column-sharded QKV weights). Scores are computed TRANSPOSED
    (keys on partitions, queries on free) so the softmaxed probabilities
    feed the P@V matmul directly with no on-chip transposes of P. The
    softmax denominator comes for free from a ones-column appended to V.
  - A per-batch AllToAll (bf16) re-shards the attention output from
    head-sharded to row-sharded; the output projection then runs with the
    full Wproj.T per core plus bias. Core i returns rows
    {256i..256i+256} of each batch; the host reassembles the full output.

Walrus constraint: a fused matmul carries at most ONE semaphore wait; an
explicit ldweights before each accumulation-group start gives
move_matmul_waits_to_ldweights a place to park extra waits.
"""

import sys

for _p in ("/opt/trn_rl_repo", "/opt/pypackages"):
    if _p not in sys.path:
        sys.path.append(_p)

import numpy as np
import ml_dtypes

import concourse.bass as bass
import concourse.mybir as mybir
import concourse.tile as tile
from concourse import bacc
from concourse.bass_utils import run_bass_kernel_spmd

P = 128
CORES = 8
B, N, C = 2, 2048, 1024
H, D = 16, 64
R = B * N          # 4096 total rows
HL = H // CORES    # 2 heads per core
DL = HL * D        # 128 head dims per core
RO = R // CORES    # 512 output rows per core
RB = RO // B       # 256 rows per (core, batch)
NKC = N // P       # 16 key chunks of 128 per batch
NQC = N // 512     # 4 query chunks of 512 per batch
CK = C // P        # 8 contraction chunks of 128
SCALE = D ** -0.5  # 0.125

F32 = mybir.dt.float32
BF16 = mybir.dt.bfloat16

# A2A chunking per batch: batch0 one collective (hidden under batch1's
# attention); batch1 in three (rows qc0+qc1 | qc2 | qc3) so only the last
# 128KB collective is exposed. Entries: (start_row, n_rows) within batch.
A2A_CHUNKS = (
    ((0, N),),
    ((0, N // 2), (N // 2, N // 4), (3 * N // 4, N // 4)),
)
NH_B = tuple(len(c) for c in A2A_CHUNKS)


def build_nc():
    nc = bacc.Bacc("TRN2", target_bir_lowering=False, debug=False,
                   num_devices=CORES)

    xT_d = nc.declare_dram_parameter("xT", [C, R], BF16, isOutput=False)
    wq_d = nc.declare_dram_parameter("wqT", [C, DL], BF16, isOutput=False)
    wk_d = nc.declare_dram_parameter("wkT", [C, DL], BF16, isOutput=False)
    wv_d = nc.declare_dram_parameter("wvT", [C, DL], BF16, isOutput=False)
    wp_d = nc.declare_dram_parameter("wpT", [C, C], BF16, isOutput=False)
    bp_d = nc.declare_dram_parameter("bproj", [C], F32, isOutput=False)
    out_d = nc.declare_dram_parameter("out", [RO, C], F32, isOutput=True)

    with tile.TileContext(nc) as tc:
        build_kernel(tc, xT_d, wq_d, wk_d, wv_d, wp_d, bp_d, out_d)

    nc.compile()
    return nc


def build_kernel(tc, xT_d, wq_d, wk_d, wv_d, wp_d, bp_d, out_d):
    nc = tc.nc
    EXP = mybir.ActivationFunctionType.Exp

    with (
        tc.tile_pool(name="persist", bufs=1) as persist,
        tc.tile_pool(name="expp", bufs=3) as expp,
        tc.tile_pool(name="small", bufs=2) as small,
        tc.tile_pool(name="ypool", bufs=3) as ypool,
        tc.tile_pool(name="pA", bufs=2, space="PSUM") as pA,
        tc.tile_pool(name="pB", bufs=2, space="PSUM") as pB,
        tc.tile_pool(name="dramp", bufs=1, space="DRAM") as dramp,
    ):
        # ---------------- persistent SBUF tensors ----------------
        xT_sb = persist.tile([P, CK, R], BF16, name="xT_sb")
        wq_sb = persist.tile([P, CK, DL], BF16, name="wq_sb")
        wk_sb = persist.tile([P, CK, DL], BF16, name="wk_sb")
        wv_sb = persist.tile([P, CK, DL], BF16, name="wv_sb")
        wp_sb = persist.tile([P, CK, C], BF16, name="wp_sb")
        bias_sb = persist.tile([P, C], F32, name="bias_sb")
        qT_sb = persist.tile([P, R], BF16, name="qT_sb")
        kT_sb = persist.tile([P, R], BF16, name="kT_sb")
        vT_sb = persist.tile([P, R], BF16, name="vT_sb")
        # vaug[:, idx(b,h,kc), :]: [keys=128, D+1]; col D holds ones
        vaug_sb = persist.tile([P, B * HL * NKC, D + 1], BF16, name="vaug_sb")
        oT0_sb = persist.tile([D, R], BF16, name="oT0_sb")
        oT1_sb = persist.tile([D, R], BF16, name="oT1_sb")
        # gathered layout per (b,half): [c_in_part, src_core, BLK_b rows]
        oTg_sb = persist.tile([P, B, CORES, 256], BF16, name="oTg_sb")
        warm_sb = persist.tile([P, 512], BF16, name="warm_sb")
        ones_sb = persist.tile([1, D], BF16, name="ones_sb")

        # one A2A per (batch, half-batch): blocks [core, 128 cin, BLK_b rows]
        a2a_in = {}
        a2a_out = {}
        for b in range(B):
            for h2, (st0, nr) in enumerate(A2A_CHUNKS[b]):
                blk = nr // CORES
                a2a_in[(b, h2)] = dramp.tile([CORES, DL, blk], BF16,
                                             name=f"a2a_in{b}_{h2}")
                a2a_out[(b, h2)] = dramp.tile([CORES, DL, blk], BF16,
                                              name=f"a2a_out{b}_{h2}")

        def vidx(b, h, kc):
            return (b * HL + h) * NKC + kc

        # ---------------- constants / input DMAs ----------------
        nc.vector.memset(vaug_sb[:, :, D], 1.0)
        nc.vector.memset(warm_sb, 0.0)
        nc.vector.memset(ones_sb, 1.0)

        # PE warmup while input DMAs land: ~18 back-to-back matmuls push the
        # HAM clock gate to 8/8 before real work starts
        wps = pA.tile([P, 1024], F32, tag="big", name="wps")
        for i in range(18):
            nc.tensor.matmul(wps[:, 0:512], lhsT=warm_sb[:, 0:128],
                             rhs=warm_sb, start=(i == 0), stop=(i == 17))

        nc.sync.dma_start(out=wq_sb,
                          in_=wq_d.ap().rearrange("(o p) d -> p o d", p=P))
        nc.sync.dma_start(out=wk_sb,
                          in_=wk_d.ap().rearrange("(o p) d -> p o d", p=P))
        nc.sync.dma_start(out=wv_sb,
                          in_=wv_d.ap().rearrange("(o p) d -> p o d", p=P))

        xT_ap = xT_d.ap().rearrange("(o p) n -> p o n", p=P)
        for rc in range(R // 512):
            sl = slice(rc * 512, (rc + 1) * 512)
            nc.sync.dma_start(out=xT_sb[:, :, sl], in_=xT_ap[:, :, sl])

        bias_bcast = bass.AP(tensor=bp_d, offset=0, ap=[[0, P], [1, C]])
        nc.gpsimd.dma_start(out=bias_sb, in_=bias_bcast)

        # Wproj is only needed in the epilogue — load it last
        nc.sync.dma_start(out=wp_sb,
                          in_=wp_d.ap().rearrange("(o p) c -> p o c", p=P))

        # ---------------- QKV projections ----------------
        for rc in range(R // 512):
            b, rcl = divmod(rc, NQC)
            sl = slice(rc * 512, (rc + 1) * 512)

            for w_sb, dst in (
                (wq_sb, qT_sb),
                (wk_sb, kT_sb),
                (wv_sb, vT_sb),
            ):
                ps = pA.tile([P, 1024], F32, tag="big", name="ps")
                for o in range(CK):
                    if o == 0:
                        nc.tensor.ldweights(w_sb[:, o])
                    nc.tensor.matmul(ps[:, 0:512], lhsT=w_sb[:, o],
                                     rhs=xT_sb[:, o, sl],
                                     start=(o == 0), stop=(o == CK - 1))
                nc.vector.tensor_copy(out=dst[:, sl], in_=ps[:, 0:512])

            # transpose V into natural [keys, d] layout (both heads at once)
            for t in range(4):
                kcol = rc * 512 + t * 128
                kc_b = rcl * 4 + t
                vtr = expp.tile([P, P], BF16, tag="vtr", name="vtr")
                nc.scalar.dma_start_transpose(vtr, vT_sb[:, kcol:kcol + P])
                nc.vector.tensor_copy(out=vaug_sb[:, vidx(b, 0, kc_b), 0:D],
                                      in_=vtr[:, 0:D])
                nc.vector.tensor_copy(out=vaug_sb[:, vidx(b, 1, kc_b), 0:D],
                                      in_=vtr[:, D:2 * D])

        # ---------------- attention + A2A per batch ----------------
        last_attn = [None]

        def issue_a2a(b, half):
            st0, nr = A2A_CHUNKS[b][half]
            blk = nr // CORES
            base = b * N + st0
            span = CORES * blk
            src0 = oT0_sb[:, base:base + span].rearrange(
                "d (j r) -> d j r", j=CORES)
            src1 = oT1_sb[:, base:base + span].rearrange(
                "d (j r) -> d j r", j=CORES)
            nc.sync.dma_start(
                out=a2a_in[(b, half)][:, 0:D, :].rearrange("j d r -> d j r"),
                in_=src0)
            nc.sync.dma_start(
                out=a2a_in[(b, half)][:, D:2 * D, :].rearrange("j d r -> d j r"),
                in_=src1)
            nc.gpsimd.collective_compute(
                "AllToAll",
                mybir.AluOpType.bypass,
                replica_groups=[list(range(CORES))],
                ins=[a2a_in[(b, half)].opt()],
                outs=[a2a_out[(b, half)].opt()],
            )

        for b in range(B):
            for qc in range(NQC):
                qsl = slice(b * N + qc * 512, b * N + (qc + 1) * 512)
                otF = pB.tile([P, 1024], F32, tag="ot", name="otF")

                def s_pair(kc, qsl=qsl, b=b):
                    kst = b * N + kc * P
                    stF = pA.tile([P, 1024], F32, tag="big", name="stF")
                    for h in range(HL):
                        hsl = slice(h * D, (h + 1) * D)
                        if kc == 0:
                            nc.tensor.ldweights(kT_sb[hsl, kst:kst + P],
                                                tile_position=(h * D, 0))
                        nc.tensor.matmul(stF[:, h * 512:(h + 1) * 512],
                                         lhsT=kT_sb[hsl, kst:kst + P],
                                         rhs=qT_sb[hsl, qsl],
                                         start=True, stop=True)
                    return stF

                def exp_pv(kc, stF, otF=otF, b=b):
                    exF = expp.tile([P, 1024], BF16, tag="exp", name="exF")
                    nc.scalar.activation(out=exF, in_=stF, func=EXP,
                                         scale=SCALE)
                    for h in range(HL):
                        if kc == 0:
                            nc.tensor.ldweights(vaug_sb[:, vidx(b, h, kc), :])
                        mm = nc.tensor.matmul(
                            otF[0:D + 1, h * 512:(h + 1) * 512],
                            lhsT=vaug_sb[:, vidx(b, h, kc), :],
                            rhs=exF[:, h * 512:(h + 1) * 512],
                            start=(kc == 0), stop=(kc == NKC - 1))
                        last_attn[0] = mm.ins

                # software pipeline: scores one kc ahead of exp+PV
                prev = s_pair(0)
                for kc in range(1, NKC):
                    cur = s_pair(kc)
                    exp_pv(kc - 1, prev)
                    prev = cur
                exp_pv(NKC - 1, prev)

                # normalize: fast reciprocal of the denominator row,
                # partition-broadcast, then scale the numerators
                denom = small.tile([1, 1024], F32, tag="denom", name="denom")
                nc.vector.tensor_copy(out=denom, in_=otF[D:D + 1, :])
                recip = small.tile([1, 1024], F32, tag="recip", name="recip")
                nc.vector.reciprocal_approx_fast(out=recip, in_=denom)
                recb = small.tile([1, 1024], BF16, tag="recb", name="recb")
                nc.vector.tensor_copy(out=recb, in_=recip)
                # broadcast across partitions with a K=1 ones-matmul on PE,
                # landing in otF's unused partitions 64..127 (same banks,
                # disjoint partitions -- no extra PSUM needed)
                nc.tensor.matmul(otF[D:P, 0:512], lhsT=ones_sb,
                                 rhs=recb[:, 0:512], start=True, stop=True)
                nc.tensor.matmul(otF[D:P, 512:1024], lhsT=ones_sb,
                                 rhs=recb[:, 512:1024], start=True, stop=True)
                rbc = expp.tile([D, 1024], F32, tag="rbc", name="rbc")
                nc.vector.tensor_copy(out=rbc, in_=otF[D:P, :])
                nc.vector.tensor_mul(out=oT0_sb[:, qsl], in0=otF[0:D, 0:512],
                                     in1=rbc[:, 0:512])
                nc.vector.tensor_mul(out=oT1_sb[:, qsl], in0=otF[0:D, 512:],
                                     in1=rbc[:, 512:])

                done_rows = (qc + 1) * 512
                for h2, (st0, nr) in enumerate(A2A_CHUNKS[b]):
                    if st0 + nr == done_rows:
                        issue_a2a(b, h2)

        # ---------------- gather + output projection ----------------
        for b in range(B):
            for h2, (st0, nr) in enumerate(A2A_CHUNKS[b]):
                blk = nr // CORES
                boff = st0 // CORES
                nc.sync.dma_start(
                    out=oTg_sb[:, b, :, boff:boff + blk],
                    in_=a2a_out[(b, h2)].rearrange("k p r -> p k r"))
                for r2 in range(max(1, blk // P)):
                    rw = min(P, blk)
                    rsl = slice(boff + r2 * P, boff + r2 * P + rw)
                    for oc in range(C // 512):
                        osl = slice(oc * 512, (oc + 1) * 512)
                        psy = pA.tile([P, 1024], F32, tag="big", name="psy")
                        for o in range(CK):
                            if o == 0:
                                ldw = nc.tensor.ldweights(
                                    oTg_sb[:, b, o, rsl])
                                if last_attn[0] is not None:
                                    tile.add_dep_helper(
                                        ldw.ins, last_attn[0], sync=False,
                                        reason="proj after attention")
                            mm = nc.tensor.matmul(
                                psy[0:rw, 0:512],
                                lhsT=oTg_sb[:, b, o, rsl],
                                rhs=wp_sb[:, o, osl],
                                start=(o == 0), stop=(o == CK - 1))
                            if o == 0 and last_attn[0] is not None:
                                tile.add_dep_helper(
                                    mm.ins, last_attn[0], sync=False,
                                    reason="proj after attention")
                        y_sb = ypool.tile([P, 512], F32, tag="y", name="y_sb")
                        nc.vector.tensor_add(out=y_sb[0:rw],
                                             in0=psy[0:rw, 0:512],
                                             in1=bias_sb[0:rw, osl])
                        nc.sync.dma_start(
                            out=out_d.ap()[b * RB + boff + r2 * P:
                                           b * RB + boff + r2 * P + rw,
                                           osl],
                            in_=y_sb[0:rw])


_CACHE = {}


def _get_nc():
    if "nc" not in _CACHE:
        _CACHE["nc"] = build_nc()
    return _CACHE["nc"]


def make_in_maps(x, Wq, Wk, Wv, Wproj, bproj):
    bf = ml_dtypes.bfloat16
    x = np.asarray(x, dtype=np.float32).reshape(R, C)
    xT = np.ascontiguousarray(x.T).astype(bf)
    wpT = np.ascontiguousarray(np.asarray(Wproj, np.float32).T).astype(bf)
    bp = np.ascontiguousarray(np.asarray(bproj, np.float32))
    in_maps = []
    for i in range(CORES):
        hs = slice(DL * i, DL * (i + 1))
        in_maps.append({
            "xT": xT,
            "wqT": np.ascontiguousarray(np.asarray(Wq, np.float32)[hs].T).astype(bf),
            "wkT": np.ascontiguousarray(np.asarray(Wk, np.float32)[hs].T).astype(bf),
            "wvT": np.ascontiguousarray(np.asarray(Wv, np.float32)[hs].T).astype(bf),
            "wpT": wpT,
            "bproj": bp,
        })
    return in_maps


def assemble_out(results):
    # core i's rows: per batch b and A2A chunk (st0, nr): global rows
    # b*N + st0 + (nr//CORES)*i, stored at out[b*RB + st0//CORES]
    y = np.zeros((R, C), np.float32)
    for i in range(CORES):
        o = results[i]["out"]
        for b in range(B):
            for st0, nr in A2A_CHUNKS[b]:
                blk = nr // CORES
                g = b * N + st0 + blk * i
                c = b * RB + st0 // CORES
                y[g:g + blk] = o[c:c + blk]
    return y.reshape(B, N, C)


def kernel(x, Wq, Wk, Wv, Wproj, bproj):
    nc = _get_nc()
    in_maps = make_in_maps(x, Wq, Wk, Wv, Wproj, bproj)
    res = run_bass_kernel_spmd(nc, in_maps, core_ids=list(range(CORES)))
    return assemble_out(res.results)


# revision 35
# speedup vs baseline: 1.2304x; 1.2304x over previous
"""Distributed multi-head attention for 8 trn2 NeuronCores.

Strategy (Ulysses-style head-sharding):
  - Every core receives the full activations pre-transposed/cast host-side:
    xT [C, B*N] bf16. Heads are sharded 2-per-core for QKV + attention
    (Megatron column-sharded QKV weights). Scores are computed TRANSPOSED
    (keys on partitions, queries on free) so the softmaxed probabilities
    feed the P@V matmul directly with no on-chip transposes of P. The
    softmax denominator comes for free from a ones-column appended to V.
  - A per-batch AllToAll (bf16) re-shards the attention output from
    head-sharded to row-sharded; the output projection then runs with the
    full Wproj.T per core plus bias. Core i returns rows
    {256i..256i+256} of each batch; the host reassembles the full output.

Walrus constraint: a fused matmul carries at most ONE semaphore wait; an
explicit ldweights before each accumulation-group start gives
move_matmul_waits_to_ldweights a place to park extra waits.
"""

import sys

for _p in ("/opt/trn_rl_repo", "/opt/pypackages"):
    if _p not in sys.path:
        sys.path.append(_p)

import numpy as np
import ml_dtypes

import concourse.bass as bass
import concourse.mybir as mybir
import concourse.tile as tile
from concourse import bacc
from concourse.bass_utils import run_bass_kernel_spmd

P = 128
CORES = 8
B, N, C = 2, 2048, 1024
H, D = 16, 64
R = B * N          # 4096 total rows
HL = H // CORES    # 2 heads per core
DL = HL * D        # 128 head dims per core
RO = R // CORES    # 512 output rows per core
RB = RO // B       # 256 rows per (core, batch)
NKC = N // P       # 16 key chunks of 128 per batch
NQC = N // 512     # 4 query chunks of 512 per batch
CK = C // P        # 8 contraction chunks of 128
SCALE = D ** -0.5  # 0.125

F32 = mybir.dt.float32
BF16 = mybir.dt.bfloat16

# A2A chunking per batch: batch0 one collective (hidden under batch1's
# attention); batch1 in three (rows qc0+qc1 | qc2 | qc3) so only the last
# 128KB collective is exposed. Entries: (start_row, n_rows) within batch.
A2A_CHUNKS = (
    ((0, N),),
    ((0, N // 2), (N // 2, N // 2)),
)
NH_B = tuple(len(c) for c in A2A_CHUNKS)


def build_nc():
    nc = bacc.Bacc("TRN2", target_bir_lowering=False, debug=False,
                   num_devices=CORES)

    xT_d = nc.declare_dram_parameter("xT", [C, R], BF16, isOutput=False)
    wq_d = nc.declare_dram_parameter("wqT", [C, DL], BF16, isOutput=False)
    wk_d = nc.declare_dram_parameter("wkT", [C, DL], BF16, isOutput=False)
    wv_d = nc.declare_dram_parameter("wvT", [C, DL], BF16, isOutput=False)
    wp_d = nc.declare_dram_parameter("wpT", [C, C], BF16, isOutput=False)
    bp_d = nc.declare_dram_parameter("bproj", [C], F32, isOutput=False)
    out_d = nc.declare_dram_parameter("out", [RO, C], F32, isOutput=True)

    with tile.TileContext(nc) as tc:
        build_kernel(tc, xT_d, wq_d, wk_d, wv_d, wp_d, bp_d, out_d)

    nc.compile()
    return nc


def build_kernel(tc, xT_d, wq_d, wk_d, wv_d, wp_d, bp_d, out_d):
    nc = tc.nc
    EXP = mybir.ActivationFunctionType.Exp

    with (
        tc.tile_pool(name="persist", bufs=1) as persist,
        tc.tile_pool(name="expp", bufs=3) as expp,
        tc.tile_pool(name="small", bufs=2) as small,
        tc.tile_pool(name="ypool", bufs=3) as ypool,
        tc.tile_pool(name="pA", bufs=2, space="PSUM") as pA,
        tc.tile_pool(name="pB", bufs=2, space="PSUM") as pB,
        tc.tile_pool(name="dramp", bufs=1, space="DRAM") as dramp,
    ):
        # ---------------- persistent SBUF tensors ----------------
        xT_sb = persist.tile([P, CK, R], BF16, name="xT_sb")
        wq_sb = persist.tile([P, CK, DL], BF16, name="wq_sb")
        wk_sb = persist.tile([P, CK, DL], BF16, name="wk_sb")
        wv_sb = persist.tile([P, CK, DL], BF16, name="wv_sb")
        wp_sb = persist.tile([P, CK, C], BF16, name="wp_sb")
        bias_sb = persist.tile([P, C], F32, name="bias_sb")
        qT_sb = persist.tile([P, R], BF16, name="qT_sb")
        kT_sb = persist.tile([P, R], BF16, name="kT_sb")
        vT_sb = persist.tile([P, R], BF16, name="vT_sb")
        # vaug[:, idx(b,h,kc), :]: [keys=128, D+1]; col D holds ones
        vaug_sb = persist.tile([P, B * HL * NKC, D + 1], BF16, name="vaug_sb")
        oT0_sb = persist.tile([D, R], BF16, name="oT0_sb")
        oT1_sb = persist.tile([D, R], BF16, name="oT1_sb")
        # gathered layout per (b,half): [c_in_part, src_core, BLK_b rows]
        oTg_sb = persist.tile([P, B, CORES, 256], BF16, name="oTg_sb")
        warm_sb = persist.tile([P, 512], BF16, name="warm_sb")
        ones_sb = persist.tile([1, D], BF16, name="ones_sb")

        # one A2A per (batch, half-batch): blocks [core, 128 cin, BLK_b rows]
        a2a_in = {}
        a2a_out = {}
        for b in range(B):
            for h2, (st0, nr) in enumerate(A2A_CHUNKS[b]):
                blk = nr // CORES
                a2a_in[(b, h2)] = dramp.tile([CORES, DL, blk], BF16,
                                             name=f"a2a_in{b}_{h2}")
                a2a_out[(b, h2)] = dramp.tile([CORES, DL, blk], BF16,
                                              name=f"a2a_out{b}_{h2}")

        def vidx(b, h, kc):
            return (b * HL + h) * NKC + kc

        # ---------------- constants / input DMAs ----------------
        nc.vector.memset(vaug_sb[:, :, D], 1.0)
        nc.vector.memset(warm_sb, 0.0)
        nc.vector.memset(ones_sb, 1.0)

        # PE warmup while input DMAs land: ~18 back-to-back matmuls push the
        # HAM clock gate to 8/8 before real work starts
        wps = pA.tile([P, 1024], F32, tag="big", name="wps")
        for i in range(18):
            nc.tensor.matmul(wps[:, 0:512], lhsT=warm_sb[:, 0:128],
                             rhs=warm_sb, start=(i == 0), stop=(i == 17))

        nc.sync.dma_start(out=wq_sb,
                          in_=wq_d.ap().rearrange("(o p) d -> p o d", p=P))
        nc.sync.dma_start(out=wk_sb,
                          in_=wk_d.ap().rearrange("(o p) d -> p o d", p=P))
        nc.sync.dma_start(out=wv_sb,
                          in_=wv_d.ap().rearrange("(o p) d -> p o d", p=P))

        xT_ap = xT_d.ap().rearrange("(o p) n -> p o n", p=P)
        for rc in range(R // 512):
            sl = slice(rc * 512, (rc + 1) * 512)
            nc.sync.dma_start(out=xT_sb[:, :, sl], in_=xT_ap[:, :, sl])

        bias_bcast = bass.AP(tensor=bp_d, offset=0, ap=[[0, P], [1, C]])
        nc.gpsimd.dma_start(out=bias_sb, in_=bias_bcast)

        # Wproj is only needed in the epilogue — load it last
        nc.sync.dma_start(out=wp_sb,
                          in_=wp_d.ap().rearrange("(o p) c -> p o c", p=P))

        # ---------------- QKV projections ----------------
        for rc in range(R // 512):
            b, rcl = divmod(rc, NQC)
            sl = slice(rc * 512, (rc + 1) * 512)

            for w_sb, dst in (
                (wq_sb, qT_sb),
                (wk_sb, kT_sb),
                (wv_sb, vT_sb),
            ):
                ps = pA.tile([P, 1024], F32, tag="big", name="ps")
                for o in range(CK):
                    if o == 0:
                        nc.tensor.ldweights(w_sb[:, o])
                    nc.tensor.matmul(ps[:, 0:512], lhsT=w_sb[:, o],
                                     rhs=xT_sb[:, o, sl],
                                     start=(o == 0), stop=(o == CK - 1))
                nc.vector.tensor_copy(out=dst[:, sl], in_=ps[:, 0:512])

            # transpose V into natural [keys, d] layout (both heads at once)
            for t in range(4):
                kcol = rc * 512 + t * 128
                kc_b = rcl * 4 + t
                vtr = expp.tile([P, P], BF16, tag="vtr", name="vtr")
                nc.sync.dma_start_transpose(vtr, vT_sb[:, kcol:kcol + P])
                nc.vector.tensor_copy(out=vaug_sb[:, vidx(b, 0, kc_b), 0:D],
                                      in_=vtr[:, 0:D])
                nc.vector.tensor_copy(out=vaug_sb[:, vidx(b, 1, kc_b), 0:D],
                                      in_=vtr[:, D:2 * D])

        # ---------------- attention + A2A per batch ----------------
        last_attn = [None]

        def issue_a2a(b, half):
            st0, nr = A2A_CHUNKS[b][half]
            blk = nr // CORES
            base = b * N + st0
            span = CORES * blk
            src0 = oT0_sb[:, base:base + span].rearrange(
                "d (j r) -> d j r", j=CORES)
            src1 = oT1_sb[:, base:base + span].rearrange(
                "d (j r) -> d j r", j=CORES)
            nc.sync.dma_start(
                out=a2a_in[(b, half)][:, 0:D, :].rearrange("j d r -> d j r"),
                in_=src0)
            nc.sync.dma_start(
                out=a2a_in[(b, half)][:, D:2 * D, :].rearrange("j d r -> d j r"),
                in_=src1)
            nc.gpsimd.collective_compute(
                "AllToAll",
                mybir.AluOpType.bypass,
                replica_groups=[list(range(CORES))],
                ins=[a2a_in[(b, half)].opt()],
                outs=[a2a_out[(b, half)].opt()],
            )

        for b in range(B):
            for qc in range(NQC):
                qsl = slice(b * N + qc * 512, b * N + (qc + 1) * 512)
                otF = pB.tile([P, 1024], F32, tag="ot", name="otF")

                def s_pair(kc, qsl=qsl, b=b):
                    kst = b * N + kc * P
                    stF = pA.tile([P, 1024], F32, tag="big", name="stF")
                    for h in range(HL):
                        hsl = slice(h * D, (h + 1) * D)
                        if kc == 0:
                            nc.tensor.ldweights(kT_sb[hsl, kst:kst + P],
                                                tile_position=(h * D, 0))
                        nc.tensor.matmul(stF[:, h * 512:(h + 1) * 512],
                                         lhsT=kT_sb[hsl, kst:kst + P],
                                         rhs=qT_sb[hsl, qsl],
                                         start=True, stop=True)
                    return stF

                def exp_pv(kc, stF, otF=otF, b=b):
                    exF = expp.tile([P, 1024], BF16, tag="exp", name="exF")
                    nc.scalar.activation(out=exF, in_=stF, func=EXP,
                                         scale=SCALE)
                    for h in range(HL):
                        if kc == 0:
                            nc.tensor.ldweights(vaug_sb[:, vidx(b, h, kc), :])
                        mm = nc.tensor.matmul(
                            otF[0:D + 1, h * 512:(h + 1) * 512],
                            lhsT=vaug_sb[:, vidx(b, h, kc), :],
                            rhs=exF[:, h * 512:(h + 1) * 512],
                            start=(kc == 0), stop=(kc == NKC - 1))
                        last_attn[0] = mm.ins

                # software pipeline: scores one kc ahead of exp+PV
                prev = s_pair(0)
                for kc in range(1, NKC):
                    cur = s_pair(kc)
                    exp_pv(kc - 1, prev)
                    prev = cur
                exp_pv(NKC - 1, prev)

                # normalize: fast reciprocal of the denominator row,
                # partition-broadcast, then scale the numerators
                denom = small.tile([1, 1024], F32, tag="denom", name="denom")
                nc.vector.tensor_copy(out=denom, in_=otF[D:D + 1, :])
                recip = small.tile([1, 1024], F32, tag="recip", name="recip")
                nc.vector.reciprocal_approx_fast(out=recip, in_=denom)
                recb = small.tile([1, 1024], BF16, tag="recb", name="recb")
                nc.vector.tensor_copy(out=recb, in_=recip)
                # broadcast across partitions with a K=1 ones-matmul on PE,
                # landing in otF's unused partitions 64..127 (same banks,
                # disjoint partitions -- no extra PSUM needed)
                nc.tensor.matmul(otF[D:P, 0:512], lhsT=ones_sb,
                                 rhs=recb[:, 0:512], start=True, stop=True)
                nc.tensor.matmul(otF[D:P, 512:1024], lhsT=ones_sb,
                                 rhs=recb[:, 512:1024], start=True, stop=True)
                rbc = expp.tile([D, 1024], F32, tag="rbc", name="rbc")
                nc.vector.tensor_copy(out=rbc, in_=otF[D:P, :])
                nc.vector.tensor_mul(out=oT0_sb[:, qsl], in0=otF[0:D, 0:512],
                                     in1=rbc[:, 0:512])
                nc.vector.tensor_mul(out=oT1_sb[:, qsl], in0=otF[0:D, 512:],
                                     in1=rbc[:, 512:])

                done_rows = (qc + 1) * 512
                for h2, (st0, nr) in enumerate(A2A_CHUNKS[b]):
                    if st0 + nr == done_rows:
                        issue_a2a(b, h2)

        # ---------------- gather + output projection ----------------
        for b in range(B):
            for h2, (st0, nr) in enumerate(A2A_CHUNKS[b]):
                blk = nr // CORES
                boff = st0 // CORES
                nc.sync.dma_start(
                    out=oTg_sb[:, b, :, boff:boff + blk],
                    in_=a2a_out[(b, h2)].rearrange("k p r -> p k r"))
                for r2 in range(max(1, blk // P)):
                    rw = min(P, blk)
                    rsl = slice(boff + r2 * P, boff + r2 * P + rw)
                    for oc in range(C // 512):
                        osl = slice(oc * 512, (oc + 1) * 512)
                        psy = pA.tile([P, 1024], F32, tag="big", name="psy")
                        for o in range(CK):
                            if o == 0:
                                ldw = nc.tensor.ldweights(
                                    oTg_sb[:, b, o, rsl])
                                if last_attn[0] is not None:
                                    tile.add_dep_helper(
                                        ldw.ins, last_attn[0], sync=False,
                                        reason="proj after attention")
                            mm = nc.tensor.matmul(
                                psy[0:rw, 0:512],
                                lhsT=oTg_sb[:, b, o, rsl],
                                rhs=wp_sb[:, o, osl],
                                start=(o == 0), stop=(o == CK - 1))
                            if o == 0 and last_attn[0] is not None:
                                tile.add_dep_helper(
                                    mm.ins, last_attn[0], sync=False,
                                    reason="proj after attention")
                        y_sb = ypool.tile([P, 512], F32, tag="y", name="y_sb")
                        nc.vector.tensor_add(out=y_sb[0:rw],
                                             in0=psy[0:rw, 0:512],
                                             in1=bias_sb[0:rw, osl])
                        nc.sync.dma_start(
                            out=out_d.ap()[b * RB + boff + r2 * P:
                                           b * RB + boff + r2 * P + rw,
                                           osl],
                            in_=y_sb[0:rw])


_CACHE = {}


def _get_nc():
    if "nc" not in _CACHE:
        _CACHE["nc"] = build_nc()
    return _CACHE["nc"]


def make_in_maps(x, Wq, Wk, Wv, Wproj, bproj):
    bf = ml_dtypes.bfloat16
    x = np.asarray(x, dtype=np.float32).reshape(R, C)
    xT = np.ascontiguousarray(x.T).astype(bf)
    wpT = np.ascontiguousarray(np.asarray(Wproj, np.float32).T).astype(bf)
    bp = np.ascontiguousarray(np.asarray(bproj, np.float32))
    in_maps = []
    for i in range(CORES):
        hs = slice(DL * i, DL * (i + 1))
        in_maps.append({
            "xT": xT,
            "wqT": np.ascontiguousarray(np.asarray(Wq, np.float32)[hs].T).astype(bf),
            "wkT": np.ascontiguousarray(np.asarray(Wk, np.float32)[hs].T).astype(bf),
            "wvT": np.ascontiguousarray(np.asarray(Wv, np.float32)[hs].T).astype(bf),
            "wpT": wpT,
            "bproj": bp,
        })
    return in_maps


def assemble_out(results):
    # core i's rows: per batch b and A2A chunk (st0, nr): global rows
    # b*N + st0 + (nr//CORES)*i, stored at out[b*RB + st0//CORES]
    y = np.zeros((R, C), np.float32)
    for i in range(CORES):
        o = results[i]["out"]
        for b in range(B):
            for st0, nr in A2A_CHUNKS[b]:
                blk = nr // CORES
                g = b * N + st0 + blk * i
                c = b * RB + st0 // CORES
                y[g:g + blk] = o[c:c + blk]
    return y.reshape(B, N, C)


def kernel(x, Wq, Wk, Wv, Wproj, bproj):
    nc = _get_nc()
    in_maps = make_in_maps(x, Wq, Wk, Wv, Wproj, bproj)
    res = run_bass_kernel_spmd(nc, in_maps, core_ids=list(range(CORES)))
    return assemble_out(res.results)


# revision 37
# speedup vs baseline: 1.2482x; 1.0144x over previous
"""Distributed multi-head attention for 8 trn2 NeuronCores.

Strategy (Ulysses-style head-sharding):
  - Every core receives the full activations pre-transposed/cast host-side:
    xT [C, B*N] bf16. Heads are sharded 2-per-core for QKV + attention
    (Megatron column-sharded QKV weights). Scores are computed TRANSPOSED
    (keys on partitions, queries on free) so the softmaxed probabilities
    feed the P@V matmul directly with no on-chip transposes of P. The
    softmax denominator comes for free from a ones-column appended to V.
  - A per-batch AllToAll (bf16) re-shards the attention output from
    head-sharded to row-sharded; the output projection then runs with the
    full Wproj.T per core plus bias. Core i returns rows
    {256i..256i+256} of each batch; the host reassembles the full output.

Walrus constraint: a fused matmul carries at most ONE semaphore wait; an
explicit ldweights before each accumulation-group start gives
move_matmul_waits_to_ldweights a place to park extra waits.
"""

import sys

for _p in ("/opt/trn_rl_repo", "/opt/pypackages"):
    if _p not in sys.path:
        sys.path.append(_p)

import numpy as np
import ml_dtypes

import concourse.bass as bass
import concourse.mybir as mybir
import concourse.tile as tile
from concourse import bacc
from concourse.bass_utils import run_bass_kernel_spmd

P = 128
CORES = 8
B, N, C = 2, 2048, 1024
H, D = 16, 64
R = B * N          # 4096 total rows
HL = H // CORES    # 2 heads per core
DL = HL * D        # 128 head dims per core
RO = R // CORES    # 512 output rows per core
RB = RO // B       # 256 rows per (core, batch)
NKC = N // P       # 16 key chunks of 128 per batch
NQC = N // 512     # 4 query chunks of 512 per batch
CK = C // P        # 8 contraction chunks of 128
SCALE = D ** -0.5  # 0.125

F32 = mybir.dt.float32
BF16 = mybir.dt.bfloat16

# A2A chunking per batch: batch0 one collective (hidden under batch1's
# attention); batch1 in three (rows qc0+qc1 | qc2 | qc3) so only the last
# 128KB collective is exposed. Entries: (start_row, n_rows) within batch.
A2A_CHUNKS = (
    ((0, N),),
    ((0, 3 * N // 4), (3 * N // 4, N // 4)),
)
NH_B = tuple(len(c) for c in A2A_CHUNKS)


def build_nc():
    nc = bacc.Bacc("TRN2", target_bir_lowering=False, debug=False,
                   num_devices=CORES)

    xT_d = nc.declare_dram_parameter("xT", [C, R], BF16, isOutput=False)
    wq_d = nc.declare_dram_parameter("wqT", [C, DL], BF16, isOutput=False)
    wk_d = nc.declare_dram_parameter("wkT", [C, DL], BF16, isOutput=False)
    wv_d = nc.declare_dram_parameter("wvT", [C, DL], BF16, isOutput=False)
    wp_d = nc.declare_dram_parameter("wpT", [C, C], BF16, isOutput=False)
    bp_d = nc.declare_dram_parameter("bproj", [C], F32, isOutput=False)
    out_d = nc.declare_dram_parameter("out", [RO, C], F32, isOutput=True)

    with tile.TileContext(nc) as tc:
        build_kernel(tc, xT_d, wq_d, wk_d, wv_d, wp_d, bp_d, out_d)

    nc.compile()
    return nc


def build_kernel(tc, xT_d, wq_d, wk_d, wv_d, wp_d, bp_d, out_d):
    nc = tc.nc
    EXP = mybir.ActivationFunctionType.Exp

    with (
        tc.tile_pool(name="persist", bufs=1) as persist,
        tc.tile_pool(name="expp", bufs=3) as expp,
        tc.tile_pool(name="small", bufs=2) as small,
        tc.tile_pool(name="ypool", bufs=3) as ypool,
        tc.tile_pool(name="pA", bufs=2, space="PSUM") as pA,
        tc.tile_pool(name="pB", bufs=2, space="PSUM") as pB,
        tc.tile_pool(name="dramp", bufs=1, space="DRAM") as dramp,
    ):
        # ---------------- persistent SBUF tensors ----------------
        xT_sb = persist.tile([P, CK, R], BF16, name="xT_sb")
        wq_sb = persist.tile([P, CK, DL], BF16, name="wq_sb")
        wk_sb = persist.tile([P, CK, DL], BF16, name="wk_sb")
        wv_sb = persist.tile([P, CK, DL], BF16, name="wv_sb")
        wp_sb = persist.tile([P, CK, C], BF16, name="wp_sb")
        bias_sb = persist.tile([P, C], F32, name="bias_sb")
        qT_sb = persist.tile([P, R], BF16, name="qT_sb")
        kT_sb = persist.tile([P, R], BF16, name="kT_sb")
        vT_sb = persist.tile([P, R], BF16, name="vT_sb")
        # vaug[:, idx(b,h,kc), :]: [keys=128, D+1]; col D holds ones
        vaug_sb = persist.tile([P, B * HL * NKC, D + 1], BF16, name="vaug_sb")
        oT0_sb = persist.tile([D, R], BF16, name="oT0_sb")
        oT1_sb = persist.tile([D, R], BF16, name="oT1_sb")
        # gathered layout per (b,half): [c_in_part, src_core, BLK_b rows]
        oTg_sb = persist.tile([P, B, CORES, 256], BF16, name="oTg_sb")
        warm_sb = persist.tile([P, 512], BF16, name="warm_sb")
        ones_sb = persist.tile([1, D], BF16, name="ones_sb")

        # one A2A per (batch, half-batch): blocks [core, 128 cin, BLK_b rows]
        a2a_in = {}
        a2a_out = {}
        for b in range(B):
            for h2, (st0, nr) in enumerate(A2A_CHUNKS[b]):
                blk = nr // CORES
                a2a_in[(b, h2)] = dramp.tile([CORES, DL, blk], BF16,
                                             name=f"a2a_in{b}_{h2}")
                a2a_out[(b, h2)] = dramp.tile([CORES, DL, blk], BF16,
                                              name=f"a2a_out{b}_{h2}")

        def vidx(b, h, kc):
            return (b * HL + h) * NKC + kc

        # ---------------- constants / input DMAs ----------------
        nc.vector.memset(vaug_sb[:, :, D], 1.0)
        nc.vector.memset(warm_sb, 0.0)
        nc.vector.memset(ones_sb, 1.0)

        # PE warmup while input DMAs land: ~18 back-to-back matmuls push the
        # HAM clock gate to 8/8 before real work starts
        wps = pA.tile([P, 1024], F32, tag="big", name="wps")
        for i in range(18):
            nc.tensor.matmul(wps[:, 0:512], lhsT=warm_sb[:, 0:128],
                             rhs=warm_sb, start=(i == 0), stop=(i == 17))

        nc.sync.dma_start(out=wq_sb,
                          in_=wq_d.ap().rearrange("(o p) d -> p o d", p=P))
        nc.scalar.dma_start(out=wk_sb,
                            in_=wk_d.ap().rearrange("(o p) d -> p o d", p=P))
        nc.scalar.dma_start(out=wv_sb,
                            in_=wv_d.ap().rearrange("(o p) d -> p o d", p=P))

        xT_ap = xT_d.ap().rearrange("(o p) n -> p o n", p=P)
        for rc in range(R // 512):
            sl = slice(rc * 512, (rc + 1) * 512)
            nc.sync.dma_start(out=xT_sb[:, :, sl], in_=xT_ap[:, :, sl])

        bias_bcast = bass.AP(tensor=bp_d, offset=0, ap=[[0, P], [1, C]])
        nc.gpsimd.dma_start(out=bias_sb, in_=bias_bcast)

        # Wproj is only needed in the epilogue — load it last
        nc.sync.dma_start(out=wp_sb,
                          in_=wp_d.ap().rearrange("(o p) c -> p o c", p=P))

        # ---------------- QKV projections ----------------
        for rc in range(R // 512):
            b, rcl = divmod(rc, NQC)
            sl = slice(rc * 512, (rc + 1) * 512)

            for w_sb, dst in (
                (wq_sb, qT_sb),
                (wk_sb, kT_sb),
                (wv_sb, vT_sb),
            ):
                ps = pA.tile([P, 1024], F32, tag="big", name="ps")
                for o in range(CK):
                    if o == 0:
                        nc.tensor.ldweights(w_sb[:, o])
                    nc.tensor.matmul(ps[:, 0:512], lhsT=w_sb[:, o],
                                     rhs=xT_sb[:, o, sl],
                                     start=(o == 0), stop=(o == CK - 1))
                nc.vector.tensor_copy(out=dst[:, sl], in_=ps[:, 0:512])

            # transpose V into natural [keys, d] layout (both heads at once)
            for t in range(4):
                kcol = rc * 512 + t * 128
                kc_b = rcl * 4 + t
                vtr = expp.tile([P, P], BF16, tag="vtr", name="vtr")
                nc.sync.dma_start_transpose(vtr, vT_sb[:, kcol:kcol + P])
                nc.vector.tensor_copy(out=vaug_sb[:, vidx(b, 0, kc_b), 0:D],
                                      in_=vtr[:, 0:D])
                nc.vector.tensor_copy(out=vaug_sb[:, vidx(b, 1, kc_b), 0:D],
                                      in_=vtr[:, D:2 * D])

        # ---------------- attention + A2A per batch ----------------
        last_attn = [None]

        def issue_a2a(b, half):
            st0, nr = A2A_CHUNKS[b][half]
            blk = nr // CORES
            base = b * N + st0
            span = CORES * blk
            src0 = oT0_sb[:, base:base + span].rearrange(
                "d (j r) -> d j r", j=CORES)
            src1 = oT1_sb[:, base:base + span].rearrange(
                "d (j r) -> d j r", j=CORES)
            nc.sync.dma_start(
                out=a2a_in[(b, half)][:, 0:D, :].rearrange("j d r -> d j r"),
                in_=src0)
            nc.sync.dma_start(
                out=a2a_in[(b, half)][:, D:2 * D, :].rearrange("j d r -> d j r"),
                in_=src1)
            nc.gpsimd.collective_compute(
                "AllToAll",
                mybir.AluOpType.bypass,
                replica_groups=[list(range(CORES))],
                ins=[a2a_in[(b, half)].opt()],
                outs=[a2a_out[(b, half)].opt()],
            )

        for b in range(B):
            for qc in range(NQC):
                qsl = slice(b * N + qc * 512, b * N + (qc + 1) * 512)
                otF = pB.tile([P, 1024], F32, tag="ot", name="otF")

                def s_pair(kc, qsl=qsl, b=b):
                    kst = b * N + kc * P
                    stF = pA.tile([P, 1024], F32, tag="big", name="stF")
                    for h in range(HL):
                        hsl = slice(h * D, (h + 1) * D)
                        if kc == 0:
                            nc.tensor.ldweights(kT_sb[hsl, kst:kst + P],
                                                tile_position=(h * D, 0))
                        nc.tensor.matmul(stF[:, h * 512:(h + 1) * 512],
                                         lhsT=kT_sb[hsl, kst:kst + P],
                                         rhs=qT_sb[hsl, qsl],
                                         start=True, stop=True)
                    return stF

                def exp_pv(kc, stF, otF=otF, b=b):
                    exF = expp.tile([P, 1024], BF16, tag="exp", name="exF")
                    nc.scalar.activation(out=exF, in_=stF, func=EXP,
                                         scale=SCALE)
                    for h in range(HL):
                        if kc == 0:
                            nc.tensor.ldweights(vaug_sb[:, vidx(b, h, kc), :])
                        mm = nc.tensor.matmul(
                            otF[0:D + 1, h * 512:(h + 1) * 512],
                            lhsT=vaug_sb[:, vidx(b, h, kc), :],
                            rhs=exF[:, h * 512:(h + 1) * 512],
                            start=(kc == 0), stop=(kc == NKC - 1))
                        last_attn[0] = mm.ins

                # software pipeline: scores one kc ahead of exp+PV
                prev = s_pair(0)
                for kc in range(1, NKC):
                    cur = s_pair(kc)
                    exp_pv(kc - 1, prev)
                    prev = cur
                exp_pv(NKC - 1, prev)

                # normalize: fast reciprocal of the denominator row,
                # partition-broadcast, then scale the numerators
                denom = small.tile([1, 1024], F32, tag="denom", name="denom")
                nc.vector.tensor_copy(out=denom, in_=otF[D:D + 1, :])
                recip = small.tile([1, 1024], F32, tag="recip", name="recip")
                nc.vector.reciprocal_approx_fast(out=recip, in_=denom)
                recb = small.tile([1, 1024], BF16, tag="recb", name="recb")
                nc.vector.tensor_copy(out=recb, in_=recip)
                # broadcast across partitions with a K=1 ones-matmul on PE,
                # landing in otF's unused partitions 64..127 (same banks,
                # disjoint partitions -- no extra PSUM needed)
                nc.tensor.matmul(otF[D:P, 0:512], lhsT=ones_sb,
                                 rhs=recb[:, 0:512], start=True, stop=True)
                nc.tensor.matmul(otF[D:P, 512:1024], lhsT=ones_sb,
                                 rhs=recb[:, 512:1024], start=True, stop=True)
                rbc = expp.tile([D, 1024], F32, tag="rbc", name="rbc")
                nc.vector.tensor_copy(out=rbc, in_=otF[D:P, :])
                nc.vector.tensor_mul(out=oT0_sb[:, qsl], in0=otF[0:D, 0:512],
                                     in1=rbc[:, 0:512])
                nc.vector.tensor_mul(out=oT1_sb[:, qsl], in0=otF[0:D, 512:],
                                     in1=rbc[:, 512:])

                done_rows = (qc + 1) * 512
                for h2, (st0, nr) in enumerate(A2A_CHUNKS[b]):
                    if st0 + nr == done_rows:
                        issue_a2a(b, h2)
                assert N % 512 == 0

        # ---------------- gather + output projection ----------------
        for b in range(B):
            for h2, (st0, nr) in enumerate(A2A_CHUNKS[b]):
                blk = nr // CORES
                boff = st0 // CORES
                nc.sync.dma_start(
                    out=oTg_sb[:, b, :, boff:boff + blk],
                    in_=a2a_out[(b, h2)].rearrange("k p r -> p k r"))
                for r0 in range(0, blk, P):
                    rw = min(P, blk - r0)
                    rsl = slice(boff + r0, boff + r0 + rw)
                    for oc in range(C // 512):
                        osl = slice(oc * 512, (oc + 1) * 512)
                        psy = pA.tile([P, 1024], F32, tag="big", name="psy")
                        for o in range(CK):
                            if o == 0:
                                ldw = nc.tensor.ldweights(
                                    oTg_sb[:, b, o, rsl])
                                if last_attn[0] is not None:
                                    tile.add_dep_helper(
                                        ldw.ins, last_attn[0], sync=False,
                                        reason="proj after attention")
                            mm = nc.tensor.matmul(
                                psy[0:rw, 0:512],
                                lhsT=oTg_sb[:, b, o, rsl],
                                rhs=wp_sb[:, o, osl],
                                start=(o == 0), stop=(o == CK - 1))
                            if o == 0 and last_attn[0] is not None:
                                tile.add_dep_helper(
                                    mm.ins, last_attn[0], sync=False,
                                    reason="proj after attention")
                        y_sb = ypool.tile([P, 512], F32, tag="y", name="y_sb")
                        nc.vector.tensor_add(out=y_sb[0:rw],
                                             in0=psy[0:rw, 0:512],
                                             in1=bias_sb[0:rw, osl])
                        nc.sync.dma_start(
                            out=out_d.ap()[b * RB + boff + r0:
                                           b * RB + boff + r0 + rw,
                                           osl],
                            in_=y_sb[0:rw])


_CACHE = {}


def _get_nc():
    if "nc" not in _CACHE:
        _CACHE["nc"] = build_nc()
    return _CACHE["nc"]


def make_in_maps(x, Wq, Wk, Wv, Wproj, bproj):
    bf = ml_dtypes.bfloat16
    x = np.asarray(x, dtype=np.float32).reshape(R, C)
    xT = np.ascontiguousarray(x.T).astype(bf)
    wpT = np.ascontiguousarray(np.asarray(Wproj, np.float32).T).astype(bf)
    bp = np.ascontiguousarray(np.asarray(bproj, np.float32))
    in_maps = []
    for i in range(CORES):
        hs = slice(DL * i, DL * (i + 1))
        in_maps.append({
            "xT": xT,
            "wqT": np.ascontiguousarray(np.asarray(Wq, np.float32)[hs].T).astype(bf),
            "wkT": np.ascontiguousarray(np.asarray(Wk, np.float32)[hs].T).astype(bf),
            "wvT": np.ascontiguousarray(np.asarray(Wv, np.float32)[hs].T).astype(bf),
            "wpT": wpT,
            "bproj": bp,
        })
    return in_maps


def assemble_out(results):
    # core i's rows: per batch b and A2A chunk (st0, nr): global rows
    # b*N + st0 + (nr//CORES)*i, stored at out[b*RB + st0//CORES]
    y = np.zeros((R, C), np.float32)
    for i in range(CORES):
        o = results[i]["out"]
        for b in range(B):
            for st0, nr in A2A_CHUNKS[b]:
                blk = nr // CORES
                g = b * N + st0 + blk * i
                c = b * RB + st0 // CORES
                y[g:g + blk] = o[c:c + blk]
    return y.reshape(B, N, C)


def kernel(x, Wq, Wk, Wv, Wproj, bproj):
    nc = _get_nc()
    in_maps = make_in_maps(x, Wq, Wk, Wv, Wproj, bproj)
    res = run_bass_kernel_spmd(nc, in_maps, core_ids=list(range(CORES)))
    return assemble_out(res.results)


# revision 38
# speedup vs baseline: 1.2617x; 1.0109x over previous
"""Distributed multi-head attention for 8 trn2 NeuronCores.

Strategy (Ulysses-style head-sharding):
  - Every core receives the full activations pre-transposed/cast host-side:
    xT [C, B*N] bf16. Heads are sharded 2-per-core for QKV + attention
    (Megatron column-sharded QKV weights). Scores are computed TRANSPOSED
    (keys on partitions, queries on free) so the softmaxed probabilities
    feed the P@V matmul directly with no on-chip transposes of P. The
    softmax denominator comes for free from a ones-column appended to V.
  - A per-batch AllToAll (bf16) re-shards the attention output from
    head-sharded to row-sharded; the output projection then runs with the
    full Wproj.T per core plus bias. Core i returns rows
    {256i..256i+256} of each batch; the host reassembles the full output.

Walrus constraint: a fused matmul carries at most ONE semaphore wait; an
explicit ldweights before each accumulation-group start gives
move_matmul_waits_to_ldweights a place to park extra waits.
"""

import sys

for _p in ("/opt/trn_rl_repo", "/opt/pypackages"):
    if _p not in sys.path:
        sys.path.append(_p)

import numpy as np
import ml_dtypes

import concourse.bass as bass
import concourse.mybir as mybir
import concourse.tile as tile
from concourse import bacc
from concourse.bass_utils import run_bass_kernel_spmd

P = 128
CORES = 8
B, N, C = 2, 2048, 1024
H, D = 16, 64
R = B * N          # 4096 total rows
HL = H // CORES    # 2 heads per core
DL = HL * D        # 128 head dims per core
RO = R // CORES    # 512 output rows per core
RB = RO // B       # 256 rows per (core, batch)
NKC = N // P       # 16 key chunks of 128 per batch
NQC = N // 512     # 4 query chunks of 512 per batch
CK = C // P        # 8 contraction chunks of 128
SCALE = D ** -0.5  # 0.125

F32 = mybir.dt.float32
BF16 = mybir.dt.bfloat16

# A2A chunking per batch: batch0 one collective (hidden under batch1's
# attention); batch1 in three (rows qc0+qc1 | qc2 | qc3) so only the last
# 128KB collective is exposed. Entries: (start_row, n_rows) within batch.
A2A_CHUNKS = (
    ((0, 3 * N // 4), (3 * N // 4, N // 4)),
    ((0, 3 * N // 4), (3 * N // 4, N // 4)),
)
NH_B = tuple(len(c) for c in A2A_CHUNKS)


def build_nc():
    nc = bacc.Bacc("TRN2", target_bir_lowering=False, debug=False,
                   num_devices=CORES)

    xT_d = nc.declare_dram_parameter("xT", [C, R], BF16, isOutput=False)
    wq_d = nc.declare_dram_parameter("wqT", [C, DL], BF16, isOutput=False)
    wk_d = nc.declare_dram_parameter("wkT", [C, DL], BF16, isOutput=False)
    wv_d = nc.declare_dram_parameter("wvT", [C, DL], BF16, isOutput=False)
    wp_d = nc.declare_dram_parameter("wpT", [C, C], BF16, isOutput=False)
    bp_d = nc.declare_dram_parameter("bproj", [C], F32, isOutput=False)
    out_d = nc.declare_dram_parameter("out", [RO, C], F32, isOutput=True)

    with tile.TileContext(nc) as tc:
        build_kernel(tc, xT_d, wq_d, wk_d, wv_d, wp_d, bp_d, out_d)

    nc.compile()
    return nc


def build_kernel(tc, xT_d, wq_d, wk_d, wv_d, wp_d, bp_d, out_d):
    nc = tc.nc
    EXP = mybir.ActivationFunctionType.Exp

    with (
        tc.tile_pool(name="persist", bufs=1) as persist,
        tc.tile_pool(name="expp", bufs=3) as expp,
        tc.tile_pool(name="small", bufs=2) as small,
        tc.tile_pool(name="ypool", bufs=3) as ypool,
        tc.tile_pool(name="pA", bufs=2, space="PSUM") as pA,
        tc.tile_pool(name="pB", bufs=2, space="PSUM") as pB,
        tc.tile_pool(name="dramp", bufs=1, space="DRAM") as dramp,
    ):
        # ---------------- persistent SBUF tensors ----------------
        xT_sb = persist.tile([P, CK, R], BF16, name="xT_sb")
        wq_sb = persist.tile([P, CK, DL], BF16, name="wq_sb")
        wk_sb = persist.tile([P, CK, DL], BF16, name="wk_sb")
        wv_sb = persist.tile([P, CK, DL], BF16, name="wv_sb")
        wp_sb = persist.tile([P, CK, C], BF16, name="wp_sb")
        bias_sb = persist.tile([P, C], F32, name="bias_sb")
        qT_sb = persist.tile([P, R], BF16, name="qT_sb")
        kT_sb = persist.tile([P, R], BF16, name="kT_sb")
        vT_sb = persist.tile([P, R], BF16, name="vT_sb")
        # vaug[:, idx(b,h,kc), :]: [keys=128, D+1]; col D holds ones
        vaug_sb = persist.tile([P, B * HL * NKC, D + 1], BF16, name="vaug_sb")
        oT0_sb = persist.tile([D, R], BF16, name="oT0_sb")
        oT1_sb = persist.tile([D, R], BF16, name="oT1_sb")
        # gathered layout per (b,half): [c_in_part, src_core, BLK_b rows]
        oTg_sb = persist.tile([P, B, CORES, 256], BF16, name="oTg_sb")
        warm_sb = persist.tile([P, 512], BF16, name="warm_sb")
        ones_sb = persist.tile([1, D], BF16, name="ones_sb")

        # one A2A per (batch, half-batch): blocks [core, 128 cin, BLK_b rows]
        a2a_in = {}
        a2a_out = {}
        for b in range(B):
            for h2, (st0, nr) in enumerate(A2A_CHUNKS[b]):
                blk = nr // CORES
                a2a_in[(b, h2)] = dramp.tile([CORES, DL, blk], BF16,
                                             name=f"a2a_in{b}_{h2}")
                a2a_out[(b, h2)] = dramp.tile([CORES, DL, blk], BF16,
                                              name=f"a2a_out{b}_{h2}")

        def vidx(b, h, kc):
            return (b * HL + h) * NKC + kc

        # ---------------- constants / input DMAs ----------------
        nc.vector.memset(vaug_sb[:, :, D], 1.0)
        nc.vector.memset(warm_sb, 0.0)
        nc.vector.memset(ones_sb, 1.0)

        # PE warmup while input DMAs land: ~18 back-to-back matmuls push the
        # HAM clock gate to 8/8 before real work starts
        wps = pA.tile([P, 1024], F32, tag="big", name="wps")
        for i in range(18):
            nc.tensor.matmul(wps[:, 0:512], lhsT=warm_sb[:, 0:128],
                             rhs=warm_sb, start=(i == 0), stop=(i == 17))

        nc.sync.dma_start(out=wq_sb,
                          in_=wq_d.ap().rearrange("(o p) d -> p o d", p=P))
        nc.scalar.dma_start(out=wk_sb,
                            in_=wk_d.ap().rearrange("(o p) d -> p o d", p=P))
        nc.scalar.dma_start(out=wv_sb,
                            in_=wv_d.ap().rearrange("(o p) d -> p o d", p=P))

        xT_ap = xT_d.ap().rearrange("(o p) n -> p o n", p=P)
        for rc in range(R // 512):
            sl = slice(rc * 512, (rc + 1) * 512)
            nc.sync.dma_start(out=xT_sb[:, :, sl], in_=xT_ap[:, :, sl])

        bias_bcast = bass.AP(tensor=bp_d, offset=0, ap=[[0, P], [1, C]])
        nc.gpsimd.dma_start(out=bias_sb, in_=bias_bcast)

        # Wproj is only needed in the epilogue — load it last
        nc.sync.dma_start(out=wp_sb,
                          in_=wp_d.ap().rearrange("(o p) c -> p o c", p=P))

        # ---------------- QKV projections ----------------
        for rc in range(R // 512):
            b, rcl = divmod(rc, NQC)
            sl = slice(rc * 512, (rc + 1) * 512)

            for w_sb, dst in (
                (wq_sb, qT_sb),
                (wk_sb, kT_sb),
                (wv_sb, vT_sb),
            ):
                ps = pA.tile([P, 1024], F32, tag="big", name="ps")
                for o in range(CK):
                    if o == 0:
                        nc.tensor.ldweights(w_sb[:, o])
                    nc.tensor.matmul(ps[:, 0:512], lhsT=w_sb[:, o],
                                     rhs=xT_sb[:, o, sl],
                                     start=(o == 0), stop=(o == CK - 1))
                nc.vector.tensor_copy(out=dst[:, sl], in_=ps[:, 0:512])

            # transpose V into natural [keys, d] layout (both heads at once)
            for t in range(4):
                kcol = rc * 512 + t * 128
                kc_b = rcl * 4 + t
                vtr = expp.tile([P, P], BF16, tag="vtr", name="vtr")
                nc.sync.dma_start_transpose(vtr, vT_sb[:, kcol:kcol + P])
                nc.vector.tensor_copy(out=vaug_sb[:, vidx(b, 0, kc_b), 0:D],
                                      in_=vtr[:, 0:D])
                nc.vector.tensor_copy(out=vaug_sb[:, vidx(b, 1, kc_b), 0:D],
                                      in_=vtr[:, D:2 * D])

        # ---------------- attention + A2A per batch ----------------
        last_attn = [None]

        def issue_a2a(b, half):
            st0, nr = A2A_CHUNKS[b][half]
            blk = nr // CORES
            base = b * N + st0
            span = CORES * blk
            src0 = oT0_sb[:, base:base + span].rearrange(
                "d (j r) -> d j r", j=CORES)
            src1 = oT1_sb[:, base:base + span].rearrange(
                "d (j r) -> d j r", j=CORES)
            nc.sync.dma_start(
                out=a2a_in[(b, half)][:, 0:D, :].rearrange("j d r -> d j r"),
                in_=src0)
            nc.sync.dma_start(
                out=a2a_in[(b, half)][:, D:2 * D, :].rearrange("j d r -> d j r"),
                in_=src1)
            nc.gpsimd.collective_compute(
                "AllToAll",
                mybir.AluOpType.bypass,
                replica_groups=[list(range(CORES))],
                ins=[a2a_in[(b, half)].opt()],
                outs=[a2a_out[(b, half)].opt()],
            )

        for b in range(B):
            for qc in range(NQC):
                qsl = slice(b * N + qc * 512, b * N + (qc + 1) * 512)
                otF = pB.tile([P, 1024], F32, tag="ot", name="otF")

                def s_pair(kc, qsl=qsl, b=b):
                    kst = b * N + kc * P
                    stF = pA.tile([P, 1024], F32, tag="big", name="stF")
                    for h in range(HL):
                        hsl = slice(h * D, (h + 1) * D)
                        if kc == 0:
                            nc.tensor.ldweights(kT_sb[hsl, kst:kst + P],
                                                tile_position=(h * D, 0))
                        nc.tensor.matmul(stF[:, h * 512:(h + 1) * 512],
                                         lhsT=kT_sb[hsl, kst:kst + P],
                                         rhs=qT_sb[hsl, qsl],
                                         start=True, stop=True)
                    return stF

                def exp_pv(kc, stF, otF=otF, b=b):
                    exF = expp.tile([P, 1024], BF16, tag="exp", name="exF")
                    nc.scalar.activation(out=exF, in_=stF, func=EXP,
                                         scale=SCALE)
                    for h in range(HL):
                        if kc == 0:
                            nc.tensor.ldweights(vaug_sb[:, vidx(b, h, kc), :])
                        mm = nc.tensor.matmul(
                            otF[0:D + 1, h * 512:(h + 1) * 512],
                            lhsT=vaug_sb[:, vidx(b, h, kc), :],
                            rhs=exF[:, h * 512:(h + 1) * 512],
                            start=(kc == 0), stop=(kc == NKC - 1))
                        last_attn[0] = mm.ins

                # software pipeline: scores one kc ahead of exp+PV
                prev = s_pair(0)
                for kc in range(1, NKC):
                    cur = s_pair(kc)
                    exp_pv(kc - 1, prev)
                    prev = cur
                exp_pv(NKC - 1, prev)

                # normalize: fast reciprocal of the denominator row,
                # partition-broadcast, then scale the numerators
                denom = small.tile([1, 1024], F32, tag="denom", name="denom")
                nc.vector.tensor_copy(out=denom, in_=otF[D:D + 1, :])
                recip = small.tile([1, 1024], F32, tag="recip", name="recip")
                nc.vector.reciprocal_approx_fast(out=recip, in_=denom)
                recb = small.tile([1, 1024], BF16, tag="recb", name="recb")
                nc.vector.tensor_copy(out=recb, in_=recip)
                # broadcast across partitions with a K=1 ones-matmul on PE,
                # landing in otF's unused partitions 64..127 (same banks,
                # disjoint partitions -- no extra PSUM needed)
                nc.tensor.matmul(otF[D:P, 0:512], lhsT=ones_sb,
                                 rhs=recb[:, 0:512], start=True, stop=True)
                nc.tensor.matmul(otF[D:P, 512:1024], lhsT=ones_sb,
                                 rhs=recb[:, 512:1024], start=True, stop=True)
                rbc = expp.tile([D, 1024], F32, tag="rbc", name="rbc")
                nc.vector.tensor_copy(out=rbc, in_=otF[D:P, :])
                nc.vector.tensor_mul(out=oT0_sb[:, qsl], in0=otF[0:D, 0:512],
                                     in1=rbc[:, 0:512])
                nc.vector.tensor_mul(out=oT1_sb[:, qsl], in0=otF[0:D, 512:],
                                     in1=rbc[:, 512:])

                done_rows = (qc + 1) * 512
                for h2, (st0, nr) in enumerate(A2A_CHUNKS[b]):
                    if st0 + nr == done_rows:
                        issue_a2a(b, h2)
                assert N % 512 == 0

        # ---------------- gather + output projection ----------------
        for b in range(B):
            for h2, (st0, nr) in enumerate(A2A_CHUNKS[b]):
                blk = nr // CORES
                boff = st0 // CORES
                nc.sync.dma_start(
                    out=oTg_sb[:, b, :, boff:boff + blk],
                    in_=a2a_out[(b, h2)].rearrange("k p r -> p k r"))
                for r0 in range(0, blk, P):
                    rw = min(P, blk - r0)
                    rsl = slice(boff + r0, boff + r0 + rw)
                    for oc in range(C // 512):
                        osl = slice(oc * 512, (oc + 1) * 512)
                        psy = pA.tile([P, 1024], F32, tag="big", name="psy")
                        for o in range(CK):
                            if o == 0:
                                ldw = nc.tensor.ldweights(
                                    oTg_sb[:, b, o, rsl])
                                if last_attn[0] is not None:
                                    tile.add_dep_helper(
                                        ldw.ins, last_attn[0], sync=False,
                                        reason="proj after attention")
                            mm = nc.tensor.matmul(
                                psy[0:rw, 0:512],
                                lhsT=oTg_sb[:, b, o, rsl],
                                rhs=wp_sb[:, o, osl],
                                start=(o == 0), stop=(o == CK - 1))
                            if o == 0 and last_attn[0] is not None:
                                tile.add_dep_helper(
                                    mm.ins, last_attn[0], sync=False,
                                    reason="proj after attention")
                        y_sb = ypool.tile([P, 512], F32, tag="y", name="y_sb")
                        nc.vector.tensor_add(out=y_sb[0:rw],
                                             in0=psy[0:rw, 0:512],
                                             in1=bias_sb[0:rw, osl])
                        nc.sync.dma_start(
                            out=out_d.ap()[b * RB + boff + r0:
                                           b * RB + boff + r0 + rw,
                                           osl],
                            in_=y_sb[0:rw])


_CACHE = {}


def _get_nc():
    if "nc" not in _CACHE:
        _CACHE["nc"] = build_nc()
    return _CACHE["nc"]


def make_in_maps(x, Wq, Wk, Wv, Wproj, bproj):
    bf = ml_dtypes.bfloat16
    x = np.asarray(x, dtype=np.float32).reshape(R, C)
    xT = np.ascontiguousarray(x.T).astype(bf)
    wpT = np.ascontiguousarray(np.asarray(Wproj, np.float32).T).astype(bf)
    bp = np.ascontiguousarray(np.asarray(bproj, np.float32))
    in_maps = []
    for i in range(CORES):
        hs = slice(DL * i, DL * (i + 1))
        in_maps.append({
            "xT": xT,
            "wqT": np.ascontiguousarray(np.asarray(Wq, np.float32)[hs].T).astype(bf),
            "wkT": np.ascontiguousarray(np.asarray(Wk, np.float32)[hs].T).astype(bf),
            "wvT": np.ascontiguousarray(np.asarray(Wv, np.float32)[hs].T).astype(bf),
            "wpT": wpT,
            "bproj": bp,
        })
    return in_maps


def assemble_out(results):
    # core i's rows: per batch b and A2A chunk (st0, nr): global rows
    # b*N + st0 + (nr//CORES)*i, stored at out[b*RB + st0//CORES]
    y = np.zeros((R, C), np.float32)
    for i in range(CORES):
        o = results[i]["out"]
        for b in range(B):
            for st0, nr in A2A_CHUNKS[b]:
                blk = nr // CORES
                g = b * N + st0 + blk * i
                c = b * RB + st0 // CORES
                y[g:g + blk] = o[c:c + blk]
    return y.reshape(B, N, C)


def kernel(x, Wq, Wk, Wv, Wproj, bproj):
    nc = _get_nc()
    in_maps = make_in_maps(x, Wq, Wk, Wv, Wproj, bproj)
    res = run_bass_kernel_spmd(nc, in_maps, core_ids=list(range(CORES)))
    return assemble_out(res.results)
